# Initial kernel scaffold
#
"""T5-style encoder block (RMSNorm -> MHA w/ relative bias -> residual ->
RMSNorm -> FFN -> residual) on 8 Trainium2 NeuronCores.

Sharding: core c handles batch b = c // 4 and query-chunk qc = c % 4
(512 queries). Each core computes K/V for its batch's full sequence
(replicated within the 4-core batch group) and attention + FFN for its
512-token chunk. No collectives.

On-chip layout is feature-major ("xT" = [D, L]): activations live with
the feature dim on SBUF partitions so projection matmuls need no
transposes. Attention scores are computed transposed ([k, q]) so the
softmax probabilities feed the attn@V matmul directly; the softmax
denominator comes from an extra ones-column appended per head to the
token-major V tiles (row 64 of the [65, 512] attn output PSUM = sum of
probs). Softmax skips the max-subtraction (scores are bounded ~ +-25
for this distribution, safe in fp32). The T5 relative-position bias is
Toeplitz in (k - q), so each core gets a host-built shifted-diagonal
matrix C[h, p, m] = bias_diag[h, p - m + 3967 - 512*qc]; the bias tile
for any (head, k-tile) is just a column slice of C.

Matmuls run in bf16 (fp32 PSUM accumulation); norms/softmax stay fp32.
"""

import math
import numpy as np
from ml_dtypes import bfloat16

import concourse.bass as bass
import concourse.bacc as bacc
import concourse.mybir as mybir
from concourse import tile
from concourse.bass_utils import run_bass_kernel_spmd

AFT = mybir.ActivationFunctionType
F32, BF = mybir.dt.float32, mybir.dt.bfloat16

B, L, D, H, HD, DFF = 2, 2048, 1024, 16, 64, 4096
NUM_BUCKETS, MAX_DISTANCE = 32, 128
CH = 512          # tokens per core (query chunk)
ND = D // 128     # 8 feature tiles
NF = DFF // 128   # 32 dff tiles
NKT = L // 128    # 16 key-token tiles
NC_ = 8           # cores
CW = 2432         # width of the shifted bias matrix C
VW = H * 65       # 1040: V token tiles, 65 cols/head (64 vals + ones)
EPS = 1e-6

_CACHE = {}


def _build_program():
    nc = bacc.Bacc("TRN2", target_bir_lowering=False, debug=False, num_devices=NC_)

    xT = nc.dram_tensor("xT", [D, L], F32, kind="ExternalInput").ap()
    xTq = nc.dram_tensor("xTq", [D, CH], F32, kind="ExternalInput").ap()
    wq = nc.dram_tensor("wq", [ND, ND, 128, 128], BF, kind="ExternalInput").ap()
    wk = nc.dram_tensor("wk", [ND, ND, 128, 128], BF, kind="ExternalInput").ap()
    wv = nc.dram_tensor("wv", [ND, 128, D], BF, kind="ExternalInput").ap()
    wo = nc.dram_tensor("wo", [ND, ND, 128, 128], BF, kind="ExternalInput").ap()
    w1 = nc.dram_tensor("w1", [NF, ND, 128, 128], BF, kind="ExternalInput").ap()
    w2 = nc.dram_tensor("w2", [ND, NF, 128, 128], BF, kind="ExternalInput").ap()
    g1 = nc.dram_tensor("g1", [D, 1], F32, kind="ExternalInput").ap()
    g2 = nc.dram_tensor("g2", [D, 1], F32, kind="ExternalInput").ap()
    cb = nc.dram_tensor("cb", [H, 128, CW], BF, kind="ExternalInput").ap()
    outT = nc.dram_tensor("outT", [D, CH], F32, kind="ExternalOutput").ap()

    with tile.TileContext(nc) as tc:
        with tc.tile_pool(name="persist", bufs=1) as pp:
            ones = pp.tile([128, 1], F32, tag="ones", name="ones")
            nc.gpsimd.memset(ones[:], 1.0)
            g1s = pp.tile([128, ND], F32, tag="g1s", name="g1s")
            g2s = pp.tile([128, ND], F32, tag="g2s", name="g2s")
            nc.sync.dma_start(g1s[:], g1.rearrange("(i p) o -> p (i o)", p=128))
            nc.sync.dma_start(g2s[:], g2.rearrange("(i p) o -> p (i o)", p=128))
            hTq = [pp.tile([128, CH], BF, tag=f"hTq{i}", name=f"hTq{i}") for i in range(ND)]
            qT = [pp.tile([128, CH], BF, tag=f"qT{j}", name=f"qT{j}") for j in range(ND)]

            with tc.tile_pool(name="kv", bufs=1) as kvp:
                kT = [kvp.tile([128, L], BF, tag=f"kT{j}", name=f"kT{j}") for j in range(ND)]
                vt = [kvp.tile([128, VW], BF, tag=f"vt{t}", name=f"vt{t}") for t in range(NKT)]

                # ---------------- phase 0/1: rmsnorm + Q/K/V projections
                with tc.tile_pool(name="ph01", bufs=1) as hp, \
                     tc.tile_pool(name="xs", bufs=3) as xsp, \
                     tc.tile_pool(name="sq", bufs=2) as sqp, \
                     tc.tile_pool(name="ws", bufs=4) as wsp, \
                     tc.tile_pool(name="ev", bufs=2) as evp, \
                     tc.tile_pool(name="mm", bufs=4, space="PSUM") as mmp:

                    hT = [hp.tile([128, L], BF, tag=f"hT{i}", name=f"hT{i}") for i in range(ND)]
                    wv_sb = [hp.tile([128, D], BF, tag=f"wv{i}", name=f"wv{i}") for i in range(ND)]
                    S = hp.tile([128, L], F32, tag="S", name="S")
                    Srow = hp.tile([1, L], F32, tag="Srow", name="Srow")
                    Sq = hp.tile([128, CH], F32, tag="Sq", name="Sq")
                    Sqrow = hp.tile([1, CH], F32, tag="Sqrow", name="Sqrow")

                    # variance over full L, in 512-token chunks
                    for c in range(4):
                        vps = mmp.tile([1, 512], F32, tag="var", name=f"var{c}", bufs=2)
                        for i in range(ND):
                            xt = xsp.tile([128, 512], F32, tag="xs", name=f"x{c}_{i}")
                            nc.sync.dma_start(xt[:], xT[128 * i:128 * (i + 1), 512 * c:512 * (c + 1)])
                            sq = sqp.tile([128, 512], F32, tag="sq", name=f"sq{c}_{i}")
                            nc.vector.tensor_mul(sq[:], xt[:], xt[:])
                            nc.tensor.matmul(vps[:], ones[:], sq[:], start=(i == 0), stop=(i == ND - 1))
                        std = evp.tile([1, 512], F32, tag="std", name=f"std{c}")
                        nc.scalar.activation(std[:], vps[:], AFT.Sqrt, bias=EPS, scale=1.0 / D)
                        nc.vector.reciprocal(Srow[:, 512 * c:512 * (c + 1)], std[:])
                    nc.gpsimd.partition_broadcast(S[:], Srow[:])

                    # h = x * S * g1   (bf16, feature-major, full L)
                    for i in range(ND):
                        for c in range(4):
                            xt = xsp.tile([128, 512], F32, tag="xs", name=f"hx{i}_{c}")
                            nc.sync.dma_start(xt[:], xT[128 * i:128 * (i + 1), 512 * c:512 * (c + 1)])
                            t2 = sqp.tile([128, 512], F32, tag="sq", name=f"hm{i}_{c}")
                            nc.vector.tensor_mul(t2[:], xt[:], S[:, 512 * c:512 * (c + 1)])
                            nc.scalar.activation(hT[i][:, 512 * c:512 * (c + 1)], t2[:], AFT.Copy,
                                                 scale=g1s[:, i:i + 1])

                    # rmsnorm of the query chunk (qc offset differs per core,
                    # so it arrives as its own input xTq)
                    vpsq = mmp.tile([1, 512], F32, tag="var", name="varq", bufs=2)
                    for i in range(ND):
                        xt = xsp.tile([128, 512], F32, tag="xs", name=f"qx{i}")
                        nc.sync.dma_start(xt[:], xTq[128 * i:128 * (i + 1), :])
                        sq = sqp.tile([128, 512], F32, tag="sq", name=f"qsq{i}")
                        nc.vector.tensor_mul(sq[:], xt[:], xt[:])
                        nc.tensor.matmul(vpsq[:], ones[:], sq[:], start=(i == 0), stop=(i == ND - 1))
                    stdq = evp.tile([1, 512], F32, tag="std", name="stdq")
                    nc.scalar.activation(stdq[:], vpsq[:], AFT.Sqrt, bias=EPS, scale=1.0 / D)
                    nc.vector.reciprocal(Sqrow[:], stdq[:])
                    nc.gpsimd.partition_broadcast(Sq[:], Sqrow[:])
                    for i in range(ND):
                        xt = xsp.tile([128, 512], F32, tag="xs", name=f"qx2{i}")
                        nc.sync.dma_start(xt[:], xTq[128 * i:128 * (i + 1), :])
                        t2 = sqp.tile([128, 512], F32, tag="sq", name=f"qm{i}")
                        nc.vector.tensor_mul(t2[:], xt[:], Sq[:])
                        nc.scalar.activation(hTq[i][:], t2[:], AFT.Copy, scale=g1s[:, i:i + 1])

                    # K projection: kT[j] = (Wk col-block j)^T h, feature-major
                    for j in range(ND):
                        pss = [mmp.tile([128, 512], F32, tag="acc", name=f"kps{j}_{c}") for c in range(4)]
                        for i in range(ND):
                            wt = wsp.tile([128, 128], BF, tag="w", name=f"wk{j}_{i}")
                            nc.sync.dma_start(wt[:], wk[i, j])
                            for c in range(4):
                                nc.tensor.matmul(pss[c][:], wt[:], hT[i][:, 512 * c:512 * (c + 1)],
                                                 start=(i == 0), stop=(i == ND - 1))
                        for c in range(4):
                            nc.vector.tensor_copy(kT[j][:, 512 * c:512 * (c + 1)], pss[c][:])

                    # V projection: token-major tiles, 65 cols/head (64 + ones)
                    for i in range(ND):
                        nc.sync.dma_start(wv_sb[i][:], wv[i])
                    for t in range(NKT):
                        vtr = vt[t].rearrange("p (h c) -> p h c", c=65)
                        nc.gpsimd.memset(vtr[:, :, 64:65], 1.0)
                        for co in range(2):
                            ps = mmp.tile([128, 512], F32, tag="acc", name=f"vps{t}_{co}")
                            for i in range(ND):
                                nc.tensor.matmul(ps[:], hT[i][:, 128 * t:128 * (t + 1)],
                                                 wv_sb[i][:, 512 * co:512 * (co + 1)],
                                                 start=(i == 0), stop=(i == ND - 1))
                            nc.scalar.activation(vtr[:, 8 * co:8 * (co + 1), 0:64],
                                                 ps.rearrange("p (h c) -> p h c", c=64), AFT.Copy)

                    # Q projection (chunk only)
                    for j in range(ND):
                        ps = mmp.tile([128, 512], F32, tag="acc", name=f"qps{j}")
                        for i in range(ND):
                            wt = wsp.tile([128, 128], BF, tag="w", name=f"wq{j}_{i}")
                            nc.sync.dma_start(wt[:], wq[i, j])
                            nc.tensor.matmul(ps[:], wt[:], hTq[i][:], start=(i == 0), stop=(i == ND - 1))
                        nc.vector.tensor_copy(qT[j][:], ps[:])

                # ---------------- phases 2-5: attention, out-proj, FFN
                with tc.tile_pool(name="mid", bufs=1) as midp:
                    aoT = [midp.tile([128, CH], BF, tag=f"aoT{i}", name=f"aoT{i}") for i in range(ND)]
                    x2T = [midp.tile([128, CH], F32, tag=f"x2T{i}", name=f"x2T{i}") for i in range(ND)]
                    h2T = [midp.tile([128, CH], BF, tag=f"h2T{i}", name=f"h2T{i}") for i in range(ND)]

                    # ----- attention
                    with tc.tile_pool(name="cp", bufs=2) as cp, \
                         tc.tile_pool(name="ppool", bufs=3) as ppool, \
                         tc.tile_pool(name="rp", bufs=2) as rp, \
                         tc.tile_pool(name="scp", bufs=3, space="PSUM") as scp, \
                         tc.tile_pool(name="aop", bufs=2, space="PSUM") as aop:
                        for h in range(H):
                            j, r0 = h // 2, 64 * (h % 2)
                            ch = cp.tile([128, CW], BF, tag="C", name=f"C{h}")
                            nc.sync.dma_start(ch[:], cb[h])
                            aops = aop.tile([65, 512], F32, tag="ao", name=f"aops{h}")
                            for kt in range(NKT):
                                sc = scp.tile([128, 512], F32, tag="sc", name=f"sc{h}_{kt}")
                                nc.tensor.matmul(sc[:], kT[j][r0:r0 + 64, 128 * kt:128 * (kt + 1)],
                                                 qT[j][r0:r0 + 64, :], start=True, stop=True)
                                nc.vector.tensor_add(sc[:], sc[:], ch[:, 1920 - 128 * kt:2432 - 128 * kt])
                                p = ppool.tile([128, 512], BF, tag="p", name=f"p{h}_{kt}")
                                nc.scalar.activation(p[:], sc[:], AFT.Exp)
                                vtr = vt[kt].rearrange("q (hh c) -> q hh c", c=65)
                                nc.tensor.matmul(aops[:], vtr[:, h:h + 1, :], p[:],
                                                 start=(kt == 0), stop=(kt == NKT - 1))
                            rrow = rp.tile([1, 512], F32, tag="rrow", name=f"rrow{h}")
                            nc.vector.reciprocal(rrow[:], aops[64:65, :])
                            rb = rp.tile([64, 512], F32, tag="rb", name=f"rb{h}")
                            nc.gpsimd.partition_broadcast(rb[:], rrow[:])
                            nc.vector.tensor_mul(aoT[j][r0:r0 + 64, :], aops[0:64, :], rb[:])

                    # ----- output projection + residual
                    with tc.tile_pool(name="ws2", bufs=4) as wsp2, \
                         tc.tile_pool(name="xq", bufs=2) as xqp, \
                         tc.tile_pool(name="mm2", bufs=2, space="PSUM") as mm2:
                        for j in range(ND):
                            ps = mm2.tile([128, 512], F32, tag="o", name=f"ops{j}")
                            for i in range(ND):
                                wt = wsp2.tile([128, 128], BF, tag="w", name=f"wo{j}_{i}")
                                nc.sync.dma_start(wt[:], wo[i, j])
                                nc.tensor.matmul(ps[:], wt[:], aoT[i][:], start=(i == 0), stop=(i == ND - 1))
                            xqt = xqp.tile([128, 512], F32, tag="xq", name=f"xq{j}")
                            nc.sync.dma_start(xqt[:], xTq[128 * j:128 * (j + 1), :])
                            nc.vector.tensor_add(x2T[j][:], ps[:], xqt[:])

                    # ----- rmsnorm 2
                    with tc.tile_pool(name="n2", bufs=1) as n2p, \
                         tc.tile_pool(name="sq2", bufs=2) as sqp2, \
                         tc.tile_pool(name="mm3", bufs=2, space="PSUM") as mm3:
                        S2 = n2p.tile([128, CH], F32, tag="S2", name="S2")
                        S2row = n2p.tile([1, CH], F32, tag="S2row", name="S2row")
                        vps2 = mm3.tile([1, 512], F32, tag="var2", name="var2")
                        for i in range(ND):
                            sq = sqp2.tile([128, 512], F32, tag="sq", name=f"sq2_{i}")
                            nc.vector.tensor_mul(sq[:], x2T[i][:], x2T[i][:])
                            nc.tensor.matmul(vps2[:], ones[:], sq[:], start=(i == 0), stop=(i == ND - 1))
                        std2 = n2p.tile([1, 512], F32, tag="std2", name="std2")
                        nc.scalar.activation(std2[:], vps2[:], AFT.Sqrt, bias=EPS, scale=1.0 / D)
                        nc.vector.reciprocal(S2row[:], std2[:])
                        nc.gpsimd.partition_broadcast(S2[:], S2row[:])
                        for i in range(ND):
                            t2 = sqp2.tile([128, 512], F32, tag="sq", name=f"h2m{i}")
                            nc.vector.tensor_mul(t2[:], x2T[i][:], S2[:])
                            nc.scalar.activation(h2T[i][:], t2[:], AFT.Copy, scale=g2s[:, i:i + 1])

                    # ----- FFN: out += W2^T relu(W1^T h2); dout in 2 halves,
                    # u recomputed per half (saves keeping u resident)
                    with tc.tile_pool(name="w1p", bufs=4) as w1p, \
                         tc.tile_pool(name="w2p", bufs=4) as w2p, \
                         tc.tile_pool(name="ub", bufs=3) as ubp, \
                         tc.tile_pool(name="op", bufs=2) as op, \
                         tc.tile_pool(name="up", bufs=2, space="PSUM") as upp, \
                         tc.tile_pool(name="o2p", bufs=4, space="PSUM") as o2p:
                        for dh in range(2):
                            ps2 = [o2p.tile([128, 512], F32, tag="o2", name=f"o2_{dh}_{jj}") for jj in range(4)]
                            for f in range(NF):
                                ups = upp.tile([128, 512], F32, tag="u", name=f"u{dh}_{f}")
                                for i in range(ND):
                                    wt = w1p.tile([128, 128], BF, tag="w1", name=f"w1_{dh}_{f}_{i}")
                                    nc.sync.dma_start(wt[:], w1[f, i])
                                    nc.tensor.matmul(ups[:], wt[:], h2T[i][:], start=(i == 0), stop=(i == ND - 1))
                                ub = ubp.tile([128, 512], BF, tag="ub", name=f"ub{dh}_{f}")
                                nc.scalar.activation(ub[:], ups[:], AFT.Relu)
                                for jj in range(4):
                                    j = 4 * dh + jj
                                    wt2 = w2p.tile([128, 128], BF, tag="w2", name=f"w2_{dh}_{f}_{jj}")
                                    nc.sync.dma_start(wt2[:], w2[j, f])
                                    nc.tensor.matmul(ps2[jj][:], wt2[:], ub[:], start=(f == 0), stop=(f == NF - 1))
                            for jj in range(4):
                                j = 4 * dh + jj
                                ot = op.tile([128, 512], F32, tag="ot", name=f"ot{dh}_{jj}")
                                nc.vector.tensor_add(ot[:], ps2[jj][:], x2T[j][:])
                                nc.sync.dma_start(outT[128 * j:128 * (j + 1), :], ot[:])

    nc.compile()
    return nc


def _bias_diag(rel_table):
    """bias_diag[h, i] = bias for relative position d = i - (L-1), i in [0, 2L-1)."""
    d = np.arange(-(L - 1), L)
    nb = NUM_BUCKETS // 2
    buckets = (d > 0).astype(np.int64) * nb
    rpa = np.abs(d)
    max_exact = nb // 2
    is_small = rpa < max_exact
    safe = np.maximum(rpa, 1).astype(np.float32)
    large = max_exact + (
        np.log(safe / max_exact) / math.log(MAX_DISTANCE / max_exact) * (nb - max_exact)
    ).astype(np.int64)
    large = np.minimum(large, nb - 1)
    buckets = buckets + np.where(is_small, rpa, large)
    return np.ascontiguousarray(rel_table[buckets].T.astype(np.float32))  # [H, 2L-1]


def _tile2d(w, bi, bj):
    """[M, N] -> [M//bi, N//bj, bi, bj] contiguous tiles."""
    m, n = w.shape
    return np.ascontiguousarray(
        w.reshape(m // bi, bi, n // bj, bj).transpose(0, 2, 1, 3)
    )


def kernel(hidden_states, Wq, Wk, Wv, Wo, W1, W2, ln1_g, ln2_g, rel_table):
    if "nc" not in _CACHE:
        _CACHE["nc"] = _build_program()
    nc = _CACHE["nc"]

    x = np.asarray(hidden_states, dtype=np.float32)
    wq_t = _tile2d(np.asarray(Wq, dtype=bfloat16), 128, 128)
    wk_t = _tile2d(np.asarray(Wk, dtype=bfloat16), 128, 128)
    wo_t = _tile2d(np.asarray(Wo, dtype=bfloat16), 128, 128)
    wv_t = np.ascontiguousarray(np.asarray(Wv, dtype=bfloat16).reshape(ND, 128, D))
    w1_t = np.ascontiguousarray(_tile2d(np.asarray(W1, dtype=bfloat16), 128, 128).transpose(1, 0, 2, 3))
    w2_t = _tile2d(np.asarray(W2, dtype=bfloat16), 128, 128)
    g1 = np.ascontiguousarray(np.asarray(ln1_g, dtype=np.float32).reshape(D, 1))
    g2 = np.ascontiguousarray(np.asarray(ln2_g, dtype=np.float32).reshape(D, 1))

    bias_diag = _bias_diag(np.asarray(rel_table, dtype=np.float32))  # [H, 4095]
    p_idx = np.arange(128)[:, None]
    m_idx = np.arange(CW)[None, :]

    in_maps = []
    for c in range(NC_):
        b, qc = c // 4, c % 4
        xT_b = np.ascontiguousarray(x[b].T)
        xTq = np.ascontiguousarray(x[b, qc * CH:(qc + 1) * CH].T)
        idx = p_idx - m_idx + (3967 - 512 * qc)
        cb_c = np.ascontiguousarray(bias_diag[:, idx].astype(bfloat16))  # [H,128,CW]
        in_maps.append({
            "xT": xT_b, "xTq": xTq,
            "wq": wq_t, "wk": wk_t, "wv": wv_t, "wo": wo_t,
            "w1": w1_t, "w2": w2_t,
            "g1": g1, "g2": g2, "cb": cb_c,
        })

    res = run_bass_kernel_spmd(nc, in_maps, list(range(NC_)))

    out = np.empty((B, L, D), dtype=np.float32)
    for c in range(NC_):
        b, qc = c // 4, c % 4
        out[b, qc * CH:(qc + 1) * CH, :] = res.results[c]["outT"].T
    return out


# transposed-wq note: w1 is passed as [NF, ND, 128, 128] (f-major) to match
# the kernel's f-outer DMA order; _tile2d gives [ND, NF, ...] so we transpose.


# revision 8
# speedup vs baseline: 10802.3753x; 10802.3753x over previous
"""T5-style encoder block (RMSNorm -> MHA w/ relative bias -> residual ->
RMSNorm -> FFN -> residual) on 8 Trainium2 NeuronCores.

Sharding: core c handles batch b = c // 4 and query-chunk qc = c % 4
(512 queries). Each core computes K/V for its batch's full sequence
(replicated within the 4-core batch group) and attention + FFN for its
512-token chunk. No collectives.

On-chip layout is feature-major ("xT" = [D, L]): activations live with
the feature dim on SBUF partitions so projection matmuls need no
transposes. Attention scores are computed transposed ([k, q]) so the
softmax probabilities feed the attn@V matmul directly; the softmax
denominator comes from an extra ones-column appended per head to the
token-major V tiles (row 64 of the [65, 512] attn output PSUM = sum of
probs). Softmax skips the max-subtraction (scores are bounded ~ +-25
for this distribution, safe in fp32). The T5 relative-position bias is
Toeplitz in (k - q), so each core gets a host-built shifted-diagonal
matrix C[h, p, m] = bias_diag[h, p - m + 3967 - 512*qc]; the bias tile
for any (head, k-tile) is just a column slice of C.

Weights are pre-tiled on the host so every weight DMA lands 2KB+
contiguous runs per partition row (the DMA engines pay 2x below 512B).

Matmuls run in bf16 (fp32 PSUM accumulation); norms/softmax stay fp32.
"""

import math
import numpy as np
from ml_dtypes import bfloat16

import concourse.bass as bass
import concourse.bacc as bacc
import concourse.mybir as mybir
from concourse import tile
from concourse.bass_utils import run_bass_kernel_spmd

AFT = mybir.ActivationFunctionType
F32, BF = mybir.dt.float32, mybir.dt.bfloat16

B, L, D, H, HD, DFF = 2, 2048, 1024, 16, 64, 4096
NUM_BUCKETS, MAX_DISTANCE = 32, 128
CH = 512          # tokens per core (query chunk)
ND = D // 128     # 8 feature tiles
NF = DFF // 128   # 32 dff tiles
NKT = L // 128    # 16 key-token tiles
NC_ = 8           # cores
CW = 2432         # width of the shifted bias matrix C
VW = H * 65       # 1040: V token tiles, 65 cols/head (64 vals + ones)
EPS = 1e-6

_CACHE = {}


def _build_program(repeats=1):
    nc = bacc.Bacc("TRN2", target_bir_lowering=False, debug=False, num_devices=NC_)

    xT = nc.dram_tensor("xT", [D, L], F32, kind="ExternalInput").ap()
    xTq = nc.dram_tensor("xTq", [D, CH], F32, kind="ExternalInput").ap()
    # pre-tiled weights: wX[j][p, 128*i + c] = WX[128*i + p, 128*j + c]
    wq = nc.dram_tensor("wq", [ND, 128, D], BF, kind="ExternalInput").ap()
    wk = nc.dram_tensor("wk", [ND, 128, D], BF, kind="ExternalInput").ap()
    wo = nc.dram_tensor("wo", [ND, 128, D], BF, kind="ExternalInput").ap()
    wv = nc.dram_tensor("wv", [ND, 128, D], BF, kind="ExternalInput").ap()   # wv[i] = Wv[128i:+128, :]
    w1 = nc.dram_tensor("w1", [NF, 128, D], BF, kind="ExternalInput").ap()   # w1[f][p, 128i+c] = W1[128i+p, 128f+c]
    w2 = nc.dram_tensor("w2", [NF, 128, D], BF, kind="ExternalInput").ap()   # w2[f] = W2[128f:+128, :]
    cb = nc.dram_tensor("cb", [H, 128, CW], BF, kind="ExternalInput").ap()
    outT = nc.dram_tensor("outT", [D, CH], F32, kind="ExternalOutput").ap()

    with tile.TileContext(nc) as tc:
      for _rep in range(repeats):
        with tc.tile_pool(name=f"persist{_rep}", bufs=1) as pp:
            ones = pp.tile([128, 1], F32, tag="ones", name="ones")
            nc.gpsimd.memset(ones[:], 1.0)
            epsc = pp.tile([1, 1], F32, tag="epsc", name="epsc")
            nc.gpsimd.memset(epsc[:], EPS)
            hTq = [pp.tile([128, CH], BF, tag=f"hTq{i}", name=f"hTq{i}") for i in range(ND)]
            qT = [pp.tile([128, CH], BF, tag=f"qT{j}", name=f"qT{j}") for j in range(ND)]

            with tc.tile_pool(name=f"kv{_rep}", bufs=1) as kvp:
                kT = [kvp.tile([128, L], BF, tag=f"kT{j}", name=f"kT{j}") for j in range(ND)]
                vt = [kvp.tile([128, VW], BF, tag=f"vt{t}", name=f"vt{t}") for t in range(NKT)]

                # ---------------- phase 0/1: rmsnorm + Q/K/V projections
                with tc.tile_pool(name=f"ph01{_rep}", bufs=1) as hp, \
                     tc.tile_pool(name=f"xs{_rep}", bufs=9) as xsp, \
                     tc.tile_pool(name=f"sq{_rep}", bufs=2) as sqp, \
                     tc.tile_pool(name=f"ws{_rep}", bufs=3) as wsp, \
                     tc.tile_pool(name=f"ev{_rep}", bufs=2) as evp, \
                     tc.tile_pool(name=f"mm{_rep}", bufs=4, space="PSUM") as mmp:

                    hT = [hp.tile([128, L], BF, tag=f"hT{i}", name=f"hT{i}") for i in range(ND)]
                    wv_sb = [hp.tile([128, D], BF, tag=f"wv{i}", name=f"wv{i}") for i in range(ND)]
                    S = hp.tile([128, 512], F32, tag="S", name="S")

                    # norm: per 512-token chunk, keep the 8 x-tiles resident so
                    # the squares pass and the h-mult pass share one DMA
                    def _norm_block(dst_tiles, src_col, dst_col, tagn):
                        xts = []
                        vps = mmp.tile([1, 512], F32, tag="var", name=f"var{tagn}", bufs=2)
                        for i in range(ND):
                            xt = xsp.tile([128, 512], F32, tag="xs", name=f"x{tagn}_{i}")
                            if src_col is None:
                                nc.sync.dma_start(xt[:], xTq[128 * i:128 * (i + 1), :])
                            else:
                                nc.sync.dma_start(xt[:], xT[128 * i:128 * (i + 1), src_col:src_col + 512])
                            xts.append(xt)
                            sq = sqp.tile([128, 512], F32, tag="sq", name=f"sq{tagn}_{i}")
                            nc.vector.tensor_mul(sq[:], xt[:], xt[:])
                            nc.tensor.matmul(vps[:], ones[:], sq[:], start=(i == 0), stop=(i == ND - 1))
                        std = evp.tile([1, 512], F32, tag="std", name=f"std{tagn}")
                        nc.scalar.activation(std[:], vps[:], AFT.Sqrt, bias=epsc[:], scale=1.0 / D)
                        srow = evp.tile([1, 512], F32, tag="srow", name=f"srow{tagn}")
                        nc.vector.reciprocal(srow[:], std[:])
                        nc.gpsimd.partition_broadcast(S[:], srow[:])
                        for i in range(ND):
                            nc.vector.tensor_mul(dst_tiles[i][:, dst_col:dst_col + 512], xts[i][:], S[:])

                    for c in range(4):
                        _norm_block(hT, 512 * c, 512 * c, f"c{c}")
                    _norm_block(hTq, None, 0, "q")

                    # K projection: kT[j] = (Wk col-block j)^T h, feature-major
                    for j in range(ND):
                        wt = wsp.tile([128, D], BF, tag="w", name=f"wkb{j}")
                        nc.sync.dma_start(wt[:], wk[j])
                        pss = [mmp.tile([128, 512], F32, tag="acc", name=f"kps{j}_{c}") for c in range(4)]
                        for i in range(ND):
                            for c in range(4):
                                nc.tensor.matmul(pss[c][:], wt[:, 128 * i:128 * (i + 1)],
                                                 hT[i][:, 512 * c:512 * (c + 1)],
                                                 start=(i == 0), stop=(i == ND - 1))
                        for c in range(4):
                            nc.vector.tensor_copy(kT[j][:, 512 * c:512 * (c + 1)], pss[c][:])

                    # V projection: token-major tiles, 65 cols/head (64 + ones)
                    for i in range(ND):
                        nc.sync.dma_start(wv_sb[i][:], wv[i])
                    for t in range(NKT):
                        vtr = vt[t].rearrange("p (h c) -> p h c", c=65)
                        nc.gpsimd.memset(vtr[:, :, 64:65], 1.0)
                        for co in range(2):
                            ps = mmp.tile([128, 512], F32, tag="acc", name=f"vps{t}_{co}")
                            for i in range(ND):
                                nc.tensor.matmul(ps[:], hT[i][:, 128 * t:128 * (t + 1)],
                                                 wv_sb[i][:, 512 * co:512 * (co + 1)],
                                                 start=(i == 0), stop=(i == ND - 1))
                            nc.scalar.activation(vtr[:, 8 * co:8 * (co + 1), 0:64],
                                                 ps.rearrange("p (h c) -> p h c", c=64), AFT.Copy)

                    # Q projection (chunk only)
                    for j in range(ND):
                        wt = wsp.tile([128, D], BF, tag="w", name=f"wqb{j}")
                        nc.sync.dma_start(wt[:], wq[j])
                        ps = mmp.tile([128, 512], F32, tag="acc", name=f"qps{j}")
                        for i in range(ND):
                            nc.tensor.matmul(ps[:], wt[:, 128 * i:128 * (i + 1)], hTq[i][:],
                                             start=(i == 0), stop=(i == ND - 1))
                        nc.vector.tensor_copy(qT[j][:], ps[:])

                # ---------------- phases 2-5: attention, out-proj, FFN
                with tc.tile_pool(name=f"mid{_rep}", bufs=1) as midp:
                    aoT = [midp.tile([128, CH], BF, tag=f"aoT{i}", name=f"aoT{i}") for i in range(ND)]
                    x2T = [midp.tile([128, CH], F32, tag=f"x2T{i}", name=f"x2T{i}") for i in range(ND)]
                    h2T = [midp.tile([128, CH], BF, tag=f"h2T{i}", name=f"h2T{i}") for i in range(ND)]

                    # ----- attention
                    with tc.tile_pool(name=f"cp{_rep}", bufs=2) as cp, \
                         tc.tile_pool(name=f"ppool{_rep}", bufs=3) as ppool, \
                         tc.tile_pool(name=f"rp{_rep}", bufs=2) as rp, \
                         tc.tile_pool(name=f"scp{_rep}", bufs=3, space="PSUM") as scp, \
                         tc.tile_pool(name=f"aop{_rep}", bufs=2, space="PSUM") as aop:
                        for h in range(H):
                            j, r0 = h // 2, 64 * (h % 2)
                            ch = cp.tile([128, CW], BF, tag="C", name=f"C{h}")
                            nc.sync.dma_start(ch[:], cb[h])
                            aops = aop.tile([65, 512], F32, tag="ao", name=f"aops{h}")
                            for kt in range(NKT):
                                sc = scp.tile([128, 512], F32, tag="sc", name=f"sc{h}_{kt}")
                                nc.tensor.matmul(sc[:], kT[j][r0:r0 + 64, 128 * kt:128 * (kt + 1)],
                                                 qT[j][r0:r0 + 64, :], start=True, stop=True)
                                es = ppool.tile([128, 512], BF, tag="es", name=f"es{h}_{kt}")
                                nc.scalar.activation(es[:], sc[:], AFT.Exp)
                                p = ppool.tile([128, 512], BF, tag="p", name=f"p{h}_{kt}")
                                nc.vector.tensor_mul(p[:], es[:], ch[:, 1920 - 128 * kt:2432 - 128 * kt])
                                vtr = vt[kt].rearrange("q (hh c) -> q hh c", c=65)
                                nc.tensor.matmul(aops[:], vtr[:, h:h + 1, :], p[:],
                                                 start=(kt == 0), stop=(kt == NKT - 1))
                            rrow = rp.tile([1, 512], F32, tag="rrow", name=f"rrow{h}")
                            nc.vector.reciprocal(rrow[:], aops[64:65, :])
                            rb = rp.tile([64, 512], F32, tag="rb", name=f"rb{h}")
                            nc.gpsimd.partition_broadcast(rb[:], rrow[:])
                            nc.vector.tensor_mul(aoT[j][r0:r0 + 64, :], aops[0:64, :], rb[:])

                    # ----- output projection + residual
                    with tc.tile_pool(name=f"ws2{_rep}", bufs=3) as wsp2, \
                         tc.tile_pool(name=f"xq{_rep}", bufs=2) as xqp, \
                         tc.tile_pool(name=f"mm2{_rep}", bufs=2, space="PSUM") as mm2:
                        for j in range(ND):
                            wt = wsp2.tile([128, D], BF, tag="w", name=f"wob{j}")
                            nc.sync.dma_start(wt[:], wo[j])
                            ps = mm2.tile([128, 512], F32, tag="o", name=f"ops{j}")
                            for i in range(ND):
                                nc.tensor.matmul(ps[:], wt[:, 128 * i:128 * (i + 1)], aoT[i][:],
                                                 start=(i == 0), stop=(i == ND - 1))
                            xqt = xqp.tile([128, 512], F32, tag="xq", name=f"xq{j}")
                            nc.sync.dma_start(xqt[:], xTq[128 * j:128 * (j + 1), :])
                            nc.vector.tensor_add(x2T[j][:], ps[:], xqt[:])

                    # ----- rmsnorm 2
                    with tc.tile_pool(name=f"n2{_rep}", bufs=1) as n2p, \
                         tc.tile_pool(name=f"sq2{_rep}", bufs=2) as sqp2, \
                         tc.tile_pool(name=f"mm3{_rep}", bufs=2, space="PSUM") as mm3:
                        S2 = n2p.tile([128, CH], F32, tag="S2", name="S2")
                        vps2 = mm3.tile([1, 512], F32, tag="var2", name="var2")
                        for i in range(ND):
                            sq = sqp2.tile([128, 512], F32, tag="sq", name=f"sq2_{i}")
                            nc.vector.tensor_mul(sq[:], x2T[i][:], x2T[i][:])
                            nc.tensor.matmul(vps2[:], ones[:], sq[:], start=(i == 0), stop=(i == ND - 1))
                        std2 = n2p.tile([1, 512], F32, tag="std2", name="std2")
                        nc.scalar.activation(std2[:], vps2[:], AFT.Sqrt, bias=epsc[:], scale=1.0 / D)
                        S2row = n2p.tile([1, CH], F32, tag="S2row", name="S2row")
                        nc.vector.reciprocal(S2row[:], std2[:])
                        nc.gpsimd.partition_broadcast(S2[:], S2row[:])
                        for i in range(ND):
                            nc.vector.tensor_mul(h2T[i][:], x2T[i][:], S2[:])

                    # ----- FFN: u = relu(W1^T h2) kept resident (bf16);
                    # out accumulates in PSUM over all 32 f-tiles, 4 dout
                    # tiles at a time (pass 0 overlaps u production)
                    with tc.tile_pool(name=f"uT{_rep}", bufs=1) as utp, \
                         tc.tile_pool(name=f"w1p{_rep}", bufs=3) as w1p, \
                         tc.tile_pool(name=f"w2p{_rep}", bufs=3) as w2p, \
                         tc.tile_pool(name=f"op{_rep}", bufs=2) as op, \
                         tc.tile_pool(name=f"up{_rep}", bufs=2, space="PSUM") as upp, \
                         tc.tile_pool(name=f"o2p{_rep}", bufs=4, space="PSUM") as o2p:
                        uT = [utp.tile([128, CH], BF, tag=f"uT{f}", name=f"uT{f}") for f in range(NF)]
                        for dh in range(2):
                            ps2 = [o2p.tile([128, 512], F32, tag="o2", name=f"o2_{dh}_{jj}") for jj in range(4)]
                            for f in range(NF):
                                if dh == 0:
                                    wt = w1p.tile([128, D], BF, tag="w1", name=f"w1b{f}")
                                    nc.sync.dma_start(wt[:], w1[f])
                                    ups = upp.tile([128, 512], F32, tag="u", name=f"u{f}")
                                    for i in range(ND):
                                        nc.tensor.matmul(ups[:], wt[:, 128 * i:128 * (i + 1)], h2T[i][:],
                                                         start=(i == 0), stop=(i == ND - 1))
                                    nc.scalar.activation(uT[f][:], ups[:], AFT.Relu)
                                wt2 = w2p.tile([128, 512], BF, tag="w2", name=f"w2b{dh}_{f}")
                                nc.sync.dma_start(wt2[:], w2[f][:, 512 * dh:512 * (dh + 1)])
                                for jj in range(4):
                                    nc.tensor.matmul(ps2[jj][:], wt2[:, 128 * jj:128 * (jj + 1)], uT[f][:],
                                                     start=(f == 0), stop=(f == NF - 1))
                            for jj in range(4):
                                j = 4 * dh + jj
                                ot = op.tile([128, 512], F32, tag="ot", name=f"ot{dh}_{jj}")
                                nc.vector.tensor_add(ot[:], ps2[jj][:], x2T[j][:])
                                nc.sync.dma_start(outT[128 * j:128 * (j + 1), :], ot[:])

    nc.compile()
    return nc


def _bias_diag(rel_table):
    """bias_diag[h, i] = bias for relative position d = i - (L-1), i in [0, 2L-1)."""
    d = np.arange(-(L - 1), L)
    nb = NUM_BUCKETS // 2
    buckets = (d > 0).astype(np.int64) * nb
    rpa = np.abs(d)
    max_exact = nb // 2
    is_small = rpa < max_exact
    safe = np.maximum(rpa, 1).astype(np.float32)
    large = max_exact + (
        np.log(safe / max_exact) / math.log(MAX_DISTANCE / max_exact) * (nb - max_exact)
    ).astype(np.int64)
    large = np.minimum(large, nb - 1)
    buckets = buckets + np.where(is_small, rpa, large)
    return np.ascontiguousarray(rel_table[buckets].T.astype(np.float32))  # [H, 2L-1]


def _colblocks(w):
    """[D_in, N] -> [N//128, 128, D_in] with out[j][p, 128*i + c] = w[128*i + p, 128*j + c].

    One DMA per 128-wide output column block; each SBUF partition row is a
    2KB+ contiguous run in DRAM.
    """
    din, n = w.shape
    # [i, p, j, c] -> [j, p, i, c]
    t = w.reshape(din // 128, 128, n // 128, 128).transpose(2, 1, 0, 3)
    return np.ascontiguousarray(t.reshape(n // 128, 128, din))


def kernel(hidden_states, Wq, Wk, Wv, Wo, W1, W2, ln1_g, ln2_g, rel_table):
    if "nc" not in _CACHE:
        _CACHE["nc"] = _build_program()
    nc = _CACHE["nc"]

    x = np.asarray(hidden_states, dtype=np.float32)
    g1c = np.asarray(ln1_g, dtype=np.float32)[:, None]   # fold gains into weights
    g2c = np.asarray(ln2_g, dtype=np.float32)[:, None]
    wq_t = _colblocks((np.asarray(Wq, dtype=np.float32) * g1c).astype(bfloat16))
    wk_t = _colblocks((np.asarray(Wk, dtype=np.float32) * g1c).astype(bfloat16))
    wo_t = _colblocks(np.asarray(Wo, dtype=bfloat16))
    wv_t = np.ascontiguousarray((np.asarray(Wv, dtype=np.float32) * g1c).astype(bfloat16).reshape(ND, 128, D))
    w1_t = _colblocks((np.asarray(W1, dtype=np.float32) * g2c).astype(bfloat16))
    w2_t = np.ascontiguousarray(np.asarray(W2, dtype=bfloat16).reshape(NF, 128, D))

    bias_diag = np.exp(_bias_diag(np.asarray(rel_table, dtype=np.float32)))  # [H, 4095], exp'd
    p_idx = np.arange(128)[:, None]
    m_idx = np.arange(CW)[None, :]

    in_maps = []
    for c in range(NC_):
        b, qc = c // 4, c % 4
        xT_b = np.ascontiguousarray(x[b].T)
        xTq = np.ascontiguousarray(x[b, qc * CH:(qc + 1) * CH].T)
        idx = p_idx - m_idx + (3967 - 512 * qc)
        cb_c = np.ascontiguousarray(bias_diag[:, idx].astype(bfloat16))  # [H,128,CW]
        in_maps.append({
            "xT": xT_b, "xTq": xTq,
            "wq": wq_t, "wk": wk_t, "wv": wv_t, "wo": wo_t,
            "w1": w1_t, "w2": w2_t, "cb": cb_c,
        })

    res = run_bass_kernel_spmd(nc, in_maps, list(range(NC_)))

    out = np.empty((B, L, D), dtype=np.float32)
    for c in range(NC_):
        b, qc = c // 4, c % 4
        out[b, qc * CH:(qc + 1) * CH, :] = res.results[c]["outT"].T
    return out
